# revision 6
# baseline (speedup 1.0000x reference)
"""Averaged Hausdorff loss on 8 Trainium2 NeuronCores.

Problem: set1, set2 [B=4, N=4096, D=3] fp32.
  dist[b, n, m] = ||set1[b,n] - set2[b,m]||
  out = mean_b( mean_n min_m dist + mean_m min_n dist )

Sharding: one core per (batch, orientation) pair -> exactly 8 cores.
  core 2b+0: row mins  (queries = set1[b], database = set2[b])
  core 2b+1: col mins  (queries = set2[b], database = set1[b])

Per-core kernel: with q = query point, s = database point,
  d2(q, s) = ||q||^2 + (||s||^2 - 2 q.s)
The parenthesized part is a K=4 matmul:
  lhsT = [q0, q1, q2, 1]                  (stationary, per query-tile of 128)
  rhs  = [-2 s0, -2 s1, -2 s2, ||s||^2]   (streamed over all 4096 db points)
min over the free (database) axis commutes with the +||q||^2 per-partition
constant, so the kernel only reduces the matmul output; host adds ||q||^2,
clamps at 0, takes sqrt and means.

Since K=4 << 128, four query-tiles are packed into the four 32-row PE array
strips via tile_position and run concurrently, each writing its own PSUM
bank; one VectorE tensor_reduce(min) covers all four banks per chunk.

Precision: the fp32 self-loading matmul (S3_LW) only allows ONE sync-wait
command, which the Tile-scheduled stream violates (PSUM WAR + WAW pairs), so
fp32 operands can't be used directly.  Instead each operand is split
hi + lo with hi = fp16 (11-bit mantissa) and lo = bf16 (wide exponent, no
subnormal trouble), and the product accumulated in PSUM over three matmuls:
  x.u ~= xh.uh + xh.ul + xl.uh     (drops only the ~2^-24 xl.ul term)
fp16 x bf16 products are exact in fp32 (11+8 mantissa bits < 24), so the
result carries ~2^-20 relative error -- fp32-grade for this problem.
"""

import os
import sys

import numpy as np

for _p in ("/opt/trn_rl_repo",):
    if _p not in sys.path and os.path.isdir(_p):
        sys.path.insert(0, _p)

B, N, D = 4, 4096, 3
NCORES = 8
NTILES = N // 128          # 32 query tiles of 128
NGROUPS = NTILES // 4      # 8 groups of 4 strip-packed tiles
NCHUNKS = N // 512         # 8 database chunks of 512
WCOLS = NGROUPS * 128      # 1024 stationary columns

_nc_cache = None


def _build_nc():
    """Raw-Bass pipeline (no Tile): hardware matmul instructions only carry a
    single sync-wait slot (walrus refuses to split more), so semaphore waits
    are emitted as standalone wait_ge instructions on each engine queue, and
    the WAW hazard of PSUM buffer reuse is covered transitively by the
    reduce-done semaphore (reduce k done implies unit k's matmuls done)."""
    import concourse.bass as bass
    from concourse import mybir
    from contextlib import ExitStack

    NUNITS = NCHUNKS * NGROUPS
    nc = bass.Bass("TRN2", target_bir_lowering=False, debug=False,
                   num_devices=NCORES)
    WRH = nc.dram_tensor("WRH", [128, WCOLS + N], mybir.dt.float16,
                         kind="ExternalInput").ap()
    WRL = nc.dram_tensor("WRL", [128, WCOLS + N], mybir.dt.bfloat16,
                         kind="ExternalInput").ap()
    OUT = nc.dram_tensor("OUT", [128, NUNITS * 4], mybir.dt.float32,
                         kind="ExternalOutput").ap()

    ctx = ExitStack()
    with ctx:
        wrh = ctx.enter_context(
            nc.sbuf_tensor("wrh_sb", [128, WCOLS + N], mybir.dt.float16)).ap()
        wrl = ctx.enter_context(
            nc.sbuf_tensor("wrl_sb", [128, WCOLS + N], mybir.dt.bfloat16)).ap()
        mins = ctx.enter_context(
            nc.sbuf_tensor("mins_sb", [128, NUNITS * 4], mybir.dt.float32)).ap()
        pbuf = [
            ctx.enter_context(
                nc.psum_tensor(f"p{i}", [128, 4, 512], mybir.dt.float32)).ap()
            for i in range(2)
        ]
        dmah_sem = ctx.enter_context(nc.semaphore("dmah_sem"))
        dmal_sem = ctx.enter_context(nc.semaphore("dmal_sem"))
        pe_sem = ctx.enter_context(nc.semaphore("pe_sem"))
        vec_sem = ctx.enter_context(nc.semaphore("vec_sem"))
        block = ctx.enter_context(nc.Block())

        def wslice(t, s, g):
            return t[32 * s:32 * s + 4, g * 128:(g + 1) * 128]

        def rslice(t, s, j):
            return t[32 * s:32 * s + 4, WCOLS + j * 512:WCOLS + (j + 1) * 512]

        units = [(j, g) for j in range(NCHUNKS) for g in range(NGROUPS)]

        @block.sync
        def _(sync):
            sync.dma_start(out=wrh[:], in_=WRH[:]).then_inc(dmah_sem, 16)
            sync.dma_start(out=wrl[:], in_=WRL[:]).then_inc(dmal_sem, 16)
            sync.wait_ge(vec_sem, NUNITS)
            sync.dma_start(out=OUT[:], in_=mins[:]).then_inc(dmah_sem, 16)

        @block.tensor
        def _(pe):
            pe.wait_ge(dmah_sem, 16)
            for u, (j, g) in enumerate(units):
                if u >= 2:
                    # WAR vs reduce of unit u-2 (same buffer); WAW vs unit
                    # u-2's matmuls is implied (that reduce waited on them).
                    pe.wait_ge(vec_sem, u - 1)
                p = pbuf[u % 2]
                for s in range(4):
                    pe.matmul(p[:, s, :], wslice(wrh, s, g), rslice(wrh, s, j),
                              start=True, stop=False,
                              tile_position=(32 * s, 0))
                if u == 0:
                    pe.wait_ge(dmal_sem, 16)
                for s in range(4):
                    pe.matmul(p[:, s, :], wslice(wrh, s, g), rslice(wrl, s, j),
                              start=False, stop=False,
                              tile_position=(32 * s, 0))
                for s in range(4):
                    mm = pe.matmul(p[:, s, :], wslice(wrl, s, g),
                                   rslice(wrh, s, j),
                                   start=False, stop=True,
                                   tile_position=(32 * s, 0))
                # matmuls complete in pc order; one inc on the last is enough
                mm.then_inc(pe_sem, 1)

        @block.vector
        def _(vec):
            for u in range(NUNITS):
                vec.wait_ge(pe_sem, u + 1)
                vec.tensor_reduce(
                    mins[:, u * 4:u * 4 + 4], pbuf[u % 2][:, :, :],
                    axis=mybir.AxisListType.X, op=mybir.AluOpType.min,
                ).then_inc(vec_sem, 1)

    return nc


def _get_nc():
    global _nc_cache
    if _nc_cache is None:
        _nc_cache = _build_nc()
    return _nc_cache


def _pack_core_inputs(P: np.ndarray, S: np.ndarray):
    """P: [N, 3] query points, S: [N, 3] database points.

    Returns (WRH fp16, WRL bf16), each [128, WCOLS + N]:
      W part [*, :WCOLS]: W[32 s + d, g*128 + c] = P[(4g+s)*128 + c, d]
                          (d = 3 row: hi gets 1.0, lo gets 0.0)
      R part [*, WCOLS:]: R[32 s + d, m] = -2 S[m, d]
                          (d = 3 row: ||S[m]||^2)
    """
    import ml_dtypes

    f16, bf16 = np.float16, ml_dtypes.bfloat16
    P = P.astype(np.float32)
    S = S.astype(np.float32)

    Ph = P.astype(f16)
    Pl = (P - Ph.astype(np.float32)).astype(bf16)
    U = -2.0 * S                                     # [N, 3]
    Uh = U.astype(f16)
    Ul = (U - Uh.astype(np.float32)).astype(bf16)
    s2 = (S ** 2).sum(-1)                            # [N]
    s2h = s2.astype(f16)
    s2l = (s2 - s2h.astype(np.float32)).astype(bf16)

    def pack(Wsrc, ones_val, Rsrc, r3, dt):
        W4 = np.zeros((4, 32, NGROUPS, 128), np.float32)
        W4[:, 0:3, :, :] = Wsrc.astype(np.float32).reshape(
            NGROUPS, 4, 128, 3).transpose(1, 3, 0, 2)
        W4[:, 3, :, :] = ones_val
        R4 = np.zeros((4, 32, N), np.float32)
        R4[:, 0:3, :] = Rsrc.astype(np.float32).T[None, :, :]
        R4[:, 3, :] = r3.astype(np.float32)[None, :]
        out = np.concatenate(
            [W4.reshape(128, WCOLS), R4.reshape(128, N)], axis=1)
        return np.ascontiguousarray(out.astype(dt))

    WRH = pack(Ph, 1.0, Uh, s2h, f16)
    WRL = pack(Pl, 0.0, Ul, s2l, bf16)
    return WRH, WRL


def _unpack_mins(mins: np.ndarray) -> np.ndarray:
    """mins [128, NCHUNKS*NGROUPS*4] -> per-query min over db of
    (-2 q.s + ||s||^2), indexed by query n."""
    m = mins.reshape(128, NCHUNKS, NGROUPS, 4).min(axis=1)  # [c, g, s]
    return m.transpose(1, 2, 0).reshape(N)  # n = (4g+s)*128 + c


def make_in_maps(set1: np.ndarray, set2: np.ndarray):
    """Per-core input maps + per-core query norms."""
    in_maps, qnorms = [], []
    for c in range(NCORES):
        b, ori = c // 2, c % 2
        P = set1[b] if ori == 0 else set2[b]
        S = set2[b] if ori == 0 else set1[b]
        WRH, WRL = _pack_core_inputs(P, S)
        in_maps.append({"WRH": WRH, "WRL": WRL})
        qnorms.append((P.astype(np.float32) ** 2).sum(-1))
    return in_maps, qnorms


def kernel(set1: np.ndarray, set2: np.ndarray) -> np.ndarray:
    from concourse.bass_utils import run_bass_kernel_spmd

    set1 = np.asarray(set1, dtype=np.float32)
    set2 = np.asarray(set2, dtype=np.float32)

    nc = _get_nc()
    in_maps, qnorms = make_in_maps(set1, set2)
    res = run_bass_kernel_spmd(nc, in_maps, list(range(NCORES)))
    terms = []
    for c in range(NCORES):
        raw = _unpack_mins(np.asarray(res.results[c]["OUT"]))
        d2 = np.maximum(raw + qnorms[c], 0.0).astype(np.float32)
        terms.append(np.sqrt(d2).mean(dtype=np.float32))
    total = np.mean([terms[2 * b] + terms[2 * b + 1] for b in range(B)],
                    dtype=np.float32)
    return np.array(total, dtype=np.float32)
